# revision 55
# baseline (speedup 1.0000x reference)
"""Causal selective self-attention Trainium2 kernel (8 NeuronCores).

Sharding: core c handles batch b = c//4 and heads [3g, 3g+3) where g = c%4.
The selective-S matrix (per-batch [T,T], reduced over all 12 heads) is
computed as per-core partials over the core's own 3 heads and AllReduced
across the 4 cores of each batch.

Layouts are feature-major ("transposed"): q/k are stored [head_dim, T] so
that every matmul's stationary (lhsT) and moving (rhs) operands come out
of the preceding GEMM directly, with no on-device transposes.

Math notes:
  - softmax is computed without max-subtraction: logits = scale*q.k with
    |scale*q.k| <~ 2.6, so exp(l) <= ~14 never overflows fp16, and the
    protected BOS column (FF[:,0] == 0) lower-bounds each row's Z.
  - S partials, the AllReduce, and the output are fp16; the FF cumsum
    keeps an fp32 scan state (tensor_tensor_scan semantics), so the
    T-row accumulation does not amplify the fp16 rounding of individual
    S entries (worst-case exp(-FF) error ~0.2% vs the 2e-2 gate).
  - ffst holds exp(-FF - mask) in fp16; the per-head FF subtraction is a
    DVE fp16 multiply et *= ffst after the exp, NOT an identity-matmul
    into the logit PSUM: PE is the serializing engine in the attention
    phase and the A/B on hardware showed the matmul variant ~90us slower.
  - The causal/diagonal mask rides in as -60000 baked into the FF blocks;
    exp maps it to an exact 0 in ffst, which also cleanses the garbage
    upper halves of diagonal-pair exp tiles.

Schedule notes:
  - S^T blocks are emitted per k n-super so AllReduce chunk 0 launches as
    early as possible; the v GEMM fills the collective window. st_w piece
    writes alternate sync/gpsimd queues to halve serial issue latency
    ahead of each AllReduce trigger.
  - chunk-0 scans are emitted BEFORE the v GEMM's DVE evacuations: DVE
    queue order is emission order, so the first scans start the moment AR
    chunk 0 lands instead of after the v GEMM drains.
  - FF-scan input loads alternate gpsimd/scalar queues (two issue
    streams); the sync queue would order them behind out-writes.
  - x16 loads are issued by (n-super, contraction chunk) in consumption
    order so the first q-GEMM PSUM chain starts after ~6 pieces.
  - S partials are evacuated PSUM->SBUF alternating DVE/ACT so neither
    engine serializes the S -> AllReduce path; out evacuations alternate
    DVE/ACT the same way.
"""

import numpy as np

import concourse.bass as bass
import concourse.bacc as bacc
import concourse.mybir as mybir
import concourse.tile as tile
from contextlib import ExitStack
from concourse.bass_utils import run_bass_kernel_spmd

dt = mybir.dt
AF = mybir.ActivationFunctionType
ALU = mybir.AluOpType

B, T, C, H, HD = 2, 2048, 768, 12, 64
N_CORES = 8
HPC = 3                # heads per core
D = HPC * HD           # 192 feature dims per core
DV = HPC * 65          # v feature dims incl. ones column per head
NB = T // 128          # 16 query/key blocks of 128
NS = T // 512          # 4 i-supers of 512
CC = C // 128          # 6 contraction chunks
SCALE = 1.0 / np.sqrt(HD)

S_DT = dt.float16      # S partials + AllReduce dtype (cumsum keeps fp32
                       # scan state; worst-case exp(-FF) err ~0.2% << gate)
NEGBIG = -60000.0      # causal mask additive (fp16-safe; exp -> 0)

# c16 packed-constant column offsets: maskM | maskA | ident | bva | wpA | wpB
C16_MASKM, C16_MASKA, C16_ID = 0, 128, 256
C16_ONES = 384
C16_BVA, C16_WPA = 448, 448 + DV
C16_WPB = C16_WPA + C
C16_W = C16_WPB + C

# triangular-packed S scratch: block bj holds cols i in [128*bj, T)
BLK_LEN = [T - 128 * bj for bj in range(NB)]
# contiguous DRAM chunks of NB/NCHUNKS blocks each (separate tensors =>
# collectives operate on plain contiguous buffers)
NCHUNKS = 4
BS = NB // NCHUNKS     # blocks per chunk
CHUNK_LEN = [sum(BLK_LEN[BS * k:BS * k + BS]) for k in range(NCHUNKS)]
BLK_OFF = []  # (chunk, offset within chunk)
for bj in range(NB):
    k = bj // BS
    off = sum(BLK_LEN[BS * k:bj])
    BLK_OFF.append((k, off))
# chunk c's scans are emitted just before the first i-super that needs it
# (super s needs blocks 0..4s+3); chunk 0 is emitted in phase 1
SCANS_AT = {s: [] for s in range(NS)}
for c in range(NCHUNKS):
    first_s = max(0, (c * BS - 3 + 3) // 4)  # ceil((c*BS-3)/4)
    while 4 * first_s + 3 < c * BS:
        first_s += 1
    SCANS_AT[min(first_s, NS - 1)].append(c)


def _set_nchunks(n):
    global NCHUNKS, BS, CHUNK_LEN, BLK_OFF, SCANS_AT
    NCHUNKS = n
    BS = NB // NCHUNKS
    CHUNK_LEN = [sum(BLK_LEN[BS * k:BS * k + BS]) for k in range(NCHUNKS)]
    BLK_OFF = []
    for bj in range(NB):
        k = bj // BS
        BLK_OFF.append((k, sum(BLK_LEN[BS * k:bj])))
    SCANS_AT = {s: [] for s in range(NS)}
    for c in range(NCHUNKS):
        first_s = 0
        while 4 * first_s + 3 < c * BS:
            first_s += 1
        SCANS_AT[min(first_s, NS - 1)].append(c)


_NC_CACHE = {}
NO_AR = False  # ablation: replace AllReduce with local copy (wrong numerics)
USE_EFF = True  # True: ffst=exp(-FF), DVE multiply; False: ident-matmul inject
# blocks >= IDENT_BJ_MIN keep raw -FF and use the PE ident-matmul inject.
# HW A/B: 16 (pure eFF) beat 8 by ~60us — the ident matmuls lengthen the
# qk->exp critical path in PE's in-order stream even when ffst is ready.
IDENT_BJ_MIN = 16
# attV trail distance behind the exp/mult chain. HW A/B: full-drain lag for
# supers 0-1 was ~40us WORSE (stalls the yt->proj->out chain); 2 vs 1 below.
ATTV_LAG = 2
ET_BUFS = 8   # et ring depth; A/B'd vs 12 (no win, 12 less consistent)


def build_nc(reps=1):
    key = (reps, NO_AR, USE_EFF, NCHUNKS, IDENT_BJ_MIN, ATTV_LAG, ET_BUFS)
    if key in _NC_CACHE:
        return _NC_CACHE[key]
    nc = bacc.Bacc("TRN2", target_bir_lowering=False, debug=False,
                   num_devices=N_CORES)

    # host-swizzled: one contiguous DMA each
    xT = nc.declare_dram_parameter("xT", [128, CC * T], dt.float16, isOutput=False)
    wA = nc.declare_dram_parameter("wA", [128, CC * (2 * D + DV)], dt.float16, isOutput=False)
    c16 = nc.declare_dram_parameter("c16", [128, C16_W], dt.float16, isOutput=False)
    c32 = nc.declare_dram_parameter("c32", [128, 6], dt.float32, isOutput=False)
    out = nc.declare_dram_parameter("out", [T, C], dt.float16, isOutput=True)

    ios = (xT, wA, c16, c32, out)
    with tile.TileContext(nc) as tc:
        for _rep in range(reps):
            _emit_body(nc, tc, ios)

    nc.compile()
    _NC_CACHE[key] = nc
    return nc


# band tables: band s = diagonal band i in [512s, 512s+512) of the triangle;
# block bj's band-s piece covers block-local cols [max(0,512s-128bj),
# 512(s+1)-128bj) -- exactly what attention i-super s consumes
def _band_piece(bj, s):
    a = max(0, 512 * s - 128 * bj)
    b = 512 * (s + 1) - 128 * bj
    return a, b - a  # (start_loc, plen)


BAND_OFF = {}  # (bj, s) -> col offset inside band-s tensor
BAND_LEN = []
for _s in range(NS):
    off = 0
    for _bj in range(4 * _s + 4):
        BAND_OFF[(_bj, _s)] = off
        off += _band_piece(_bj, _s)[1]
    BAND_LEN.append(off)


def _emit_band_scans(nc, s, st_r, p1sta, ffst, zeros_t, maskm_t, maska_t):
    """relu/mask + chained-cumsum scan + exp for AR band s.

    Piece (bj, s) outputs ff[a+1 .. min(b+1, L)) with initial carried from
    the previous band's last output ff[a]; exp covers [a, min(b, L)) so the
    next band's carry column stays raw."""
    for bj in range(4 * s + 4):
        L = BLK_LEN[bj]
        a, plen = _band_piece(bj, s)
        b = a + plen
        off = BAND_OFF[(bj, s)]
        sta = p1sta.tile([128, 512], S_DT, tag="sta")
        eng = nc.gpsimd if bj % 2 == 0 else nc.scalar
        eng.dma_start(sta[:, 0:plen], st_r[s][:, off:off + plen])
        nc.vector.tensor_scalar_max(sta[:, 0:plen], sta[:, 0:plen], 0.0)
        if bj == 0:
            nc.vector.memset(sta[0:1, 0:plen], 0.0)
        if a == 0:
            nc.vector.tensor_mul(sta[:, 0:128], sta[:, 0:128], maskm_t)
        ff = ffst[bj]
        if a == 0:
            nc.vector.memset(ff[:, 0:1], 0.0)
        out_hi = min(b + 1, L)
        if out_hi > a + 1:
            # state -= S  => ff holds -cumsum(S) (exclusive)
            nc.vector.tensor_tensor_scan(
                ff[:, a + 1:out_hi], zeros_t[:, 0:out_hi - 1 - a],
                sta[:, 0:out_hi - 1 - a],
                0.0 if a == 0 else ff[:, a:a + 1], ALU.add, ALU.subtract)
        if a == 0:
            # causal/diagonal mask: -60000 where i < j
            nc.vector.tensor_add(ff[:, 0:128], ff[:, 0:128], maska_t)
        if USE_EFF and bj < IDENT_BJ_MIN:
            # exp the band window only; col b (next band's carry) stays raw
            nc.scalar.activation(ff[:, a:min(b, L)], ff[:, a:min(b, L)],
                                 AF.Exp)


def _emit_body(nc, tc, ios):
    (xT, wA, c16, c32, out) = ios
    with ExitStack() as ctx:
        dram = ctx.enter_context(tc.tile_pool(name="dram", bufs=1, space="DRAM"))
        st_w = [dram.tile([128, BAND_LEN[k]], S_DT, name=f"stw{k}", tag=f"stw{k}") for k in range(NS)]
        st_r = [dram.tile([128, BAND_LEN[k]], S_DT, name=f"str{k}", tag=f"str{k}") for k in range(NS)]

        # ---- long-lived SBUF tensors ----
        persist = ctx.enter_context(tc.tile_pool(name="persist", bufs=1))
        # q/k feature-major fp16 (m0: dims 0..128 = heads 0,1; m1: head 2)
        qT = [persist.tile([128, T], dt.float16, name="qT0", tag="qT0"),
              persist.tile([64, T], dt.float16, name="qT1", tag="qT1")]
        kT = [persist.tile([128, T], dt.float16, name="kT0", tag="kT0"),
              persist.tile([64, T], dt.float16, name="kT1", tag="kT1")]
        # v (token-major) incl. ones col per head: block tb at cols [tb*DV, ...)
        vaug = persist.tile([128, NB * DV], dt.float16, tag="vaug")
        c16_t = persist.tile([128, C16_W], dt.float16, tag="c16")
        c32_t = persist.tile([128, 6], dt.float32, tag="c32")
        zeros_t = persist.tile([128, T], S_DT, tag="zeros")
        # -FF^T per j-block, fp16, lives through phase 2 (4.5 MB)
        ffst = [persist.tile([128, BLK_LEN[bj]], dt.float16, name=f"ffst{bj}", tag=f"ffst{bj}")
                for bj in range(NB)]

        nc.sync.dma_start(c16_t[:], c16[:])
        nc.sync.dma_start(c32_t[:], c32[:])
        nc.vector.memset(zeros_t[:], 0.0)
        maskm_t = c16_t[:, C16_MASKM:C16_MASKM + 128]
        maska_t = c16_t[:, C16_MASKA:C16_MASKA + 128]
        ident_t = c16_t[:, C16_ID:C16_ID + 128]
        bva_t = c16_t[:, C16_BVA:C16_BVA + DV]
        wp_t = [c16_t[:, C16_WPA:C16_WPA + C], c16_t[0:64, C16_WPB:C16_WPB + C]]
        bq_t, bk_t, selv_t = c32_t[:, 0:2], c32_t[:, 2:4], c32_t[:, 4:6]

        MS = [(0, 128), (128, 64)]  # (dim offset, size) of the two m-tiles

        # phase-2 SBUF pools live at ctx level: if they shared addresses with
        # x16/w they would inherit a wait on the v GEMM (last x16 reader) and
        # stall the s=0 attention chain behind it.
        p1sta = ctx.enter_context(tc.tile_pool(name="p1sta", bufs=4))
        # et ring depth bounds how many qk+exp pair-chains PE/ACT can run
        # ahead of the (FF-gated) DVE multiplies during the AllReduce wait
        p2sb = ctx.enter_context(tc.tile_pool(name="p2sb", bufs=ET_BUFS))
        p2y = ctx.enter_context(tc.tile_pool(name="p2y", bufs=2))
        p2o = ctx.enter_context(tc.tile_pool(name="p2o", bufs=2))

        # ================= phase 0/1: qkv GEMMs + S partials =================
        with tc.tile_pool(name="p0", bufs=1) as p0, \
             tc.tile_pool(name="p0psum", bufs=2, space="PSUM") as p0ps, \
             tc.tile_pool(name="p1ps", bufs=4, space="PSUM") as p1ps, \
             tc.tile_pool(name="vps", bufs=2, space="PSUM") as vps, \
             tc.tile_pool(name="p1st", bufs=3) as p1st:
            x16 = p0.tile([128, CC * T], dt.float16, tag="x16")
            w_t = p0.tile([128, CC * (2 * D + DV)], dt.float16, tag="w")
            wq_t = w_t[:, 0:CC * D]
            wk_t = w_t[:, CC * D:2 * CC * D]
            wv_t = w_t[:, 2 * CC * D:]
            # load order = consumption order: wq, then x by (n-super major,
            # contraction chunk minor) so the first q-GEMM PSUM chain starts
            # after ~6 small pieces instead of the whole 3.1MB; chunked
            # 128KB-256KB pieces spray across the 16 DMA engines on HW
            nc.sync.dma_start(w_t[:, 0:CC * D], wA[:, 0:CC * D])
            for n in range(4):
                if n == 2:
                    nc.sync.dma_start(w_t[:, CC * D:2 * CC * D],
                                      wA[:, CC * D:2 * CC * D])
                for c in range(CC):
                    lo = c * T + n * 512
                    nc.sync.dma_start(x16[:, lo:lo + 512], xT[:, lo:lo + 512])
            nc.sync.dma_start(w_t[:, 2 * CC * D:], wA[:, 2 * CC * D:])

            # fused per-super loop: q super s + k super s + S band s + AR
            # band s. Band s only needs q/k supers <= s, so AR band 0 (the
            # 0.33MB band attention super 0 waits on) fires ~15-20us earlier
            # than a j-block chunk AR ever could.
            qsT = [p0.tile([128, T], dt.float16, name="qsT0", tag="qsT0"),
                   p0.tile([64, T], dt.float16, name="qsT1", tag="qsT1")]
            for n in range(4):
                for mi, (mof, msz) in enumerate(MS):
                    ps = p0ps.tile([128, 512], dt.float32, tag="qk_ps")
                    for c in range(CC):
                        nc.tensor.matmul(
                            ps[:msz, :], wq_t[:, c * D + mof: c * D + mof + msz],
                            x16[:, c * T + n * 512: c * T + (n + 1) * 512],
                            start=(c == 0), stop=(c == CC - 1))
                    nc.scalar.activation(qT[mi][:, n * 512:(n + 1) * 512],
                                         ps[:msz, :], AF.Identity,
                                         bias=bq_t[:msz, mi:mi + 1])
                    nc.vector.tensor_scalar_mul(
                        qsT[mi][:, n * 512:(n + 1) * 512],
                        qT[mi][:, n * 512:(n + 1) * 512],
                        selv_t[:msz, mi:mi + 1])
                for mi, (mof, msz) in enumerate(MS):
                    ps = p0ps.tile([128, 512], dt.float32, tag="qk_ps")
                    for c in range(CC):
                        nc.tensor.matmul(
                            ps[:msz, :], wk_t[:, c * D + mof: c * D + mof + msz],
                            x16[:, c * T + n * 512: c * T + (n + 1) * 512],
                            start=(c == 0), stop=(c == CC - 1))
                    nc.scalar.activation(kT[mi][:, n * 512:(n + 1) * 512],
                                         ps[:msz, :], AF.Identity,
                                         bias=bk_t[:msz, mi:mi + 1])
                for bj in range(4 * n + 4):
                    a, plen = _band_piece(bj, n)
                    i0 = 128 * bj + a
                    off = BAND_OFF[(bj, n)]
                    ps = p1ps.tile([128, 512], dt.float32, tag="s_ps")
                    sblk = p1st.tile([128, 512], S_DT, tag="sblk")
                    nc.tensor.matmul(ps[:, :plen], kT[0][:, bj * 128:(bj + 1) * 128],
                                     qsT[0][:, i0:i0 + plen], start=True, stop=False)
                    nc.tensor.matmul(ps[:, :plen], kT[1][:, bj * 128:(bj + 1) * 128],
                                     qsT[1][:, i0:i0 + plen], start=False, stop=True)
                    # alternate the PSUM evacuation between DVE and ACT so
                    # neither engine serializes the S -> AllReduce path
                    if bj % 2 == 0:
                        nc.vector.tensor_copy(sblk[:, :plen], ps[:, :plen])
                    else:
                        nc.scalar.activation(sblk[:, :plen], ps[:, :plen],
                                             AF.Copy)
                    # alternate issue queues to halve serial issue latency
                    # ahead of the AllReduce trigger
                    weng = nc.sync if bj % 2 == 0 else nc.gpsimd
                    weng.dma_start(st_w[n][:, off:off + plen], sblk[:, :plen])
                if NO_AR:
                    nc.gpsimd.dma_start(st_r[n][:], st_w[n][:])
                else:
                    nc.gpsimd.collective_compute(
                        "AllReduce", ALU.add,
                        replica_groups=[[0, 1, 2, 3], [4, 5, 6, 7]],
                        ins=[st_w[n][:]], outs=[st_r[n][:]])

            # band-0 scans BEFORE the v GEMM's DVE evacuations: DVE queue
            # order is emission order, so this lets the first scans start the
            # moment AR band 0 lands instead of after the v GEMM drains
            _emit_band_scans(nc, 0, st_r, p1sta, ffst, zeros_t,
                             maskm_t, maska_t)

            # v: fp16 GEMM, token-major, 65-wide per-head slots (ones column
            # comes from a zero weight column + the +1 in bva)
            for tb in range(NB):
                ps = vps.tile([128, DV], dt.float32, tag="v_ps")
                for c in range(CC):
                    nc.tensor.matmul(
                        ps[:], x16[:, c * T + tb * 128: c * T + (tb + 1) * 128],
                        wv_t[:, c * DV:(c + 1) * DV],
                        start=(c == 0), stop=(c == CC - 1))
                nc.vector.tensor_add(vaug[:, tb * DV:(tb + 1) * DV], ps[:], bva_t[:])

        # ============ phase 1b + 2, interleaved per i-super ==================
        # PSUM pool order matters: ptp reuses the banks freed earliest
        # (qk/s GEMMs), pjp reuses the v banks (freed last, needed last).
        with tc.tile_pool(name="p2pt", bufs=2, space="PSUM") as ptp, \
             tc.tile_pool(name="p2yt", bufs=2, space="PSUM") as ytp, \
             tc.tile_pool(name="p2pj", bufs=1, space="PSUM") as pjp:
            for s in range(NS):
                # FF^T scans per AR band: i-super s consumes exactly band s,
                # so attention for super s starts after its band arrives
                # while later bands are still reducing.
                # (band 0 was emitted in phase 1, before the v GEMM)
                if s > 0:
                    _emit_band_scans(nc, s, st_r, p1sta, ffst, zeros_t,
                                     maskm_t, maska_t)
                yt_sb = [p2y.tile([128, 512], dt.float16, name="ytA", tag="ytA"),
                         p2y.tile([64, 512], dt.float16, name="ytB", tag="ytB")]
                for h in range(HPC):
                    # head h dims live at rows [h*64, h*64+64) of the m-tiles
                    (qsrc, qof) = (0, h * 64) if h < 2 else (1, 0)
                    yt_ps = ytp.tile([65, 512], dt.float32, tag="yt_ps")
                    # j-blocks in pairs sharing one 2-bank PSUM tile; the FF
                    # subtraction rides the accumulation as ident @ (-FF).
                    # attV matmuls are emitted one pair late so the PE stream
                    # never stalls on the exp of the pair it just produced.
                    pending = []
                    for pj in range(2 * s + 2):
                        pt = ptp.tile([128, 1024], dt.float32, tag="pt")
                        et = p2sb.tile([128, 1024], dt.float16, tag="et")
                        spans = []
                        # per-span path: eFF (exp'd ffst + DVE mult) for early
                        # blocks; ident-matmul inject for bj >= IDENT_BJ_MIN
                        for half, bj in ((0, 2 * pj), (1, 2 * pj + 1)):
                            delta = bj - 4 * s
                            ioff = 128 * delta if delta >= 0 else 0
                            npr = 512 - ioff
                            i0 = s * 512 + ioff          # global i start
                            floc = i0 - bj * 128         # col offset inside ffst[bj]
                            eff_span = USE_EFF and bj < IDENT_BJ_MIN
                            co = 512 * half
                            nc.tensor.matmul(pt[:, co:co + npr],
                                             kT[qsrc][qof:qof + 64, bj * 128:(bj + 1) * 128],
                                             qT[qsrc][qof:qof + 64, i0:i0 + npr],
                                             start=True, stop=eff_span)
                            if not eff_span:
                                nc.tensor.matmul(pt[:, co:co + npr], ident_t,
                                                 ffst[bj][:, floc:floc + npr],
                                                 start=False, stop=True)
                            spans.append((bj, ioff, npr, co))
                        if spans[0][2] == 512:   # contiguous: one exp
                            w = 512 + spans[1][2]
                            nc.scalar.activation(et[:, :w], pt[:, :w], AF.Exp)
                        else:                    # gap: exp only valid halves
                            for bj, ioff, npr, co in spans:
                                nc.scalar.activation(et[:, co:co + npr],
                                                     pt[:, co:co + npr], AF.Exp)
                        for bj, ioff, npr, co in spans:
                            if USE_EFF and bj < IDENT_BJ_MIN:
                                # masked softmax numerator: et *= exp(-FF-mask)
                                floc = s * 512 + ioff - bj * 128
                                nc.vector.tensor_mul(et[:, co:co + npr],
                                                     et[:, co:co + npr],
                                                     ffst[bj][:, floc:floc + npr])
                        pending.append((spans, et))
                        # attV trails the exp/mult chain by ATTV_LAG pairs so
                        # the PE stream rarely stalls on et
                        if len(pending) > ATTV_LAG:
                            pspans, pet = pending.pop(0)
                            for bj, ioff, npr, co in pspans:
                                vbase = bj * DV + h * 65
                                nc.tensor.matmul(yt_ps[:, ioff:512],
                                                 vaug[:, vbase:vbase + 65],
                                                 pet[:, co:co + npr],
                                                 start=(bj == 0), stop=False)
                    for pspans, pet in pending:
                        for bj, ioff, npr, co in pspans:
                            vbase = bj * DV + h * 65
                            nc.tensor.matmul(yt_ps[:, ioff:512],
                                             vaug[:, vbase:vbase + 65],
                                             pet[:, co:co + npr],
                                             start=(bj == 0), stop=(bj == 4 * s + 3))
                    # normalize: yt[d, i] * (1 / sumexp[i])
                    rs = p2sb.tile([1, 512], dt.float16, tag="rs")
                    bc_ps = ptp.tile([64, 512], dt.float32, tag="pt")
                    with nc.allow_low_precision(reason="1/Z fp16: 5e-4 rel vs 2e-2 gate"):
                        nc.vector.reciprocal(rs[:], yt_ps[64:65, :])
                        nc.tensor.matmul(bc_ps[:], c16_t[0:1, C16_ONES:C16_ONES + 64],
                                         rs[:], start=True, stop=True)
                    bc_sb = p2sb.tile([64, 512], dt.float32, tag="bc_sb")
                    nc.vector.tensor_copy(bc_sb[:], bc_ps[:])
                    (dsti, dof) = (0, h * 64) if h < 2 else (1, 0)
                    nc.vector.tensor_mul(yt_sb[dsti][dof:dof + 64, :],
                                         yt_ps[0:64, :], bc_sb[:])
                # output projection for this i-super
                for ib in range(4):
                    po = pjp.tile([128, C], dt.float32, tag="po")
                    for nof, nsz in ((0, 512), (512, 256)):
                        nc.tensor.matmul(po[:, nof:nof + nsz],
                                         yt_sb[0][:, ib * 128:(ib + 1) * 128],
                                         wp_t[0][:, nof:nof + nsz],
                                         start=True, stop=False)
                        nc.tensor.matmul(po[:, nof:nof + nsz],
                                         yt_sb[1][:, ib * 128:(ib + 1) * 128],
                                         wp_t[1][:, nof:nof + nsz],
                                         start=False, stop=True)
                    ost = p2o.tile([128, C], dt.float16, tag="ost")
                    if ib % 2 == 0:
                        nc.vector.tensor_copy(ost[:], po[:])
                    else:
                        nc.scalar.activation(ost[:], po[:], AF.Copy)
                    r0 = s * 512 + ib * 128
                    nc.sync.dma_start(out[r0:r0 + 128, :], ost[:])


def _swizzle(w, width):
    """[CC*128, width] -> [128, CC*width] with chunk c at cols [c*width, ...)."""
    return np.ascontiguousarray(
        w.reshape(CC, 128, width).transpose(1, 0, 2).reshape(128, CC * width))


def _prep_core_inputs(x, w_attn, b_attn, w_proj, b_proj, sel_w, core):
    b, g = core // 4, core % 4
    h0 = 3 * g
    rows = slice(64 * h0, 64 * (h0 + HPC))
    f32, f16 = np.float32, np.float16
    wq = (w_attn[rows, :].T * SCALE).astype(f16)                       # [768, 192]
    wk = w_attn[C + 64 * h0: C + 64 * (h0 + HPC), :].T.astype(f16)
    wv = w_attn[2 * C + 64 * h0: 2 * C + 64 * (h0 + HPC), :].T.astype(f16)
    wv_aug = np.zeros((C, DV), f16)                                    # ones-col slot
    for h in range(HPC):
        wv_aug[:, h * 65: h * 65 + 64] = wv[:, h * 64:(h + 1) * 64]
    wA = np.concatenate([_swizzle(wq, D), _swizzle(wk, D), _swizzle(wv_aug, DV)], axis=1)

    bva = np.zeros((1, DV), f32)
    for h in range(HPC):
        bva[0, h * 65: h * 65 + 64] = b_attn[2 * C + 64 * (h0 + h): 2 * C + 64 * (h0 + h + 1)]
        bva[0, h * 65 + 64] = 1.0
    c16 = np.zeros((128, C16_W), f16)
    c16[:, C16_MASKM:C16_MASKM + 128] = np.triu(np.ones((128, 128), f32), 1)
    c16[:, C16_MASKA:C16_MASKA + 128] = np.tril(np.full((128, 128), NEGBIG, f32), -1)
    c16[:, C16_ID:C16_ID + 128] = np.eye(128, dtype=f32)
    c16[:, C16_ONES:C16_ONES + 64] = 1.0
    c16[:, C16_BVA:C16_BVA + DV] = np.tile(bva, (128, 1))
    c16[:, C16_WPA:C16_WPA + C] = w_proj[:, 64 * h0: 64 * h0 + 128].T.astype(f16)
    c16[0:64, C16_WPB:C16_WPB + C] = w_proj[:, 64 * h0 + 128: 64 * h0 + 192].T.astype(f16)

    c32 = np.zeros((128, 6), f32)
    c32[:, 0] = b_attn[64 * h0: 64 * h0 + 128] * np.float32(SCALE)
    c32[0:64, 1] = b_attn[64 * (h0 + 2): 64 * (h0 + 3)] * np.float32(SCALE)
    c32[:, 2] = b_attn[C + 64 * h0: C + 64 * h0 + 128]
    c32[0:64, 3] = b_attn[C + 64 * (h0 + 2): C + 64 * (h0 + 3)]
    c32[:, 4] = np.repeat(sel_w.astype(f32)[h0:h0 + 2], HD)
    c32[0:64, 5] = np.repeat(sel_w.astype(f32)[h0 + 2:h0 + 3], HD)

    return {
        "xT": _swizzle(np.ascontiguousarray(x[b].T).astype(f16), T),
        "wA": wA,
        "c16": c16,
        "c32": c32,
    }


def kernel(x, w_attn, b_attn, w_proj, b_proj, sel_w):
    x = np.asarray(x); w_attn = np.asarray(w_attn); b_attn = np.asarray(b_attn)
    w_proj = np.asarray(w_proj); b_proj = np.asarray(b_proj); sel_w = np.asarray(sel_w)
    nc = build_nc()
    in_maps = [_prep_core_inputs(x, w_attn, b_attn, w_proj, b_proj, sel_w, c)
               for c in range(N_CORES)]
    res = run_bass_kernel_spmd(nc, in_maps, list(range(N_CORES)))
    out = np.zeros((B, T, C), np.float32)
    for c in range(N_CORES):
        out[c // 4] += res.results[c]["out"].astype(np.float32)
    out += b_proj.astype(np.float32)
    return out

